# revision 1
# baseline (speedup 1.0000x reference)
"""MoE layer (nn_MoELayer_81630148428171) as a Trainium2 Bass kernel on 8 NeuronCores.

Strategy (data-parallel tokens + streamed expert weights, sparse top-2 compute):
  - Router runs on host (jax-cpu, bitwise-identical ops to the reference) and
    determines the *sharding*: tokens are assigned to the 8 cores with a greedy
    balancer so that every (core, expert) token count is ~equal; each core gets
    exactly 1024 tokens.
  - Each core receives its tokens pre-gathered into per-expert column segments
    (feature-major, bf16) and runs all 8 experts' FFN on just its routed tokens
    (top-2 sparse => ~2048 token-expert pairs per core):
        hidden^T = gelu(w1[e].T @ xgT_seg + b1)   (PE matmuls, bf16 in, f32 acc)
        y^T      = w2[e].T @ hidden^T + b2
    y^T tiles are PE-transposed to token-major, scaled by the combine weight and
    written to an HBM slot buffer; the final per-token output is an indirect-DMA
    gather of each token's two expert contributions plus one vector add.
  - No collectives: the host concatenates the 8 disjoint per-core token slices.
"""

import math
import numpy as np
import ml_dtypes

import concourse.bacc as bacc
import concourse.mybir as mybir
import concourse.tile as tile
from concourse.bass import IndirectOffsetOnAxis
from concourse.bass_utils import run_bass_kernel_spmd
from concourse.masks import make_identity

# Problem shapes (hardcoded per contract).
B, SEQ, H = 4, 2048, 1024
T = B * SEQ
FF = 4 * H
E = 8
TOP_K = 2
N_CORES = 8
T_PER_CORE = T // N_CORES
P = 128

BF16 = mybir.dt.bfloat16
F32 = mybir.dt.float32
I32 = mybir.dt.int32
NP_BF16 = ml_dtypes.bfloat16

_PROGRAM_CACHE: dict[int, object] = {}


# ----------------------------------------------------------------------------
# Host-side routing + sharding
# ----------------------------------------------------------------------------

def _route(x_flat, router_w, router_b):
    """Top-2 routing with bitwise-identical math to the jax reference."""
    try:
        import jax
        import jax.numpy as jnp

        cpu = jax.devices("cpu")[0]

        def f(xf, w, b):
            logits = xf @ w + b
            probs = jax.nn.softmax(logits, axis=-1)
            top_values, top_indices = jax.lax.top_k(probs, TOP_K)
            top_values = top_values / jnp.sum(top_values, axis=-1,
                                              keepdims=True)
            return top_values, top_indices

        with jax.default_device(cpu):
            tv, ti = jax.jit(f)(
                jnp.asarray(x_flat), jnp.asarray(router_w),
                jnp.asarray(router_b))
        tv = np.asarray(tv)
        ti = np.asarray(ti)
    except Exception:
        # numpy fallback (f32, same tie-breaking as lax.top_k for distinct
        # values — differences only possible for exact float ties)
        logits = x_flat @ router_w + router_b
        p = np.exp(logits - logits.max(-1, keepdims=True))
        p /= p.sum(-1, keepdims=True)
        ti = np.argsort(-p, axis=-1, kind="stable")[:, :TOP_K]
        tv = np.take_along_axis(p, ti, axis=-1)
        tv = tv / tv.sum(-1, keepdims=True)
    return (
        ti[:, 0].astype(np.int64),
        ti[:, 1].astype(np.int64),
        tv[:, 0].astype(np.float32),
        tv[:, 1].astype(np.float32),
    )


def _assign_tokens(e1, e2):
    """Greedy balanced assignment of tokens to cores.

    Keeps per-(core, expert) slot counts nearly equal while giving every core
    exactly T_PER_CORE tokens.
    """
    cnt = np.zeros((N_CORES, E), np.int64)
    tok = np.zeros(N_CORES, np.int64)
    assign = np.empty(T, np.int64)
    for t in range(T):
        a, b = e1[t], e2[t]
        best = -1
        bkey = None
        for c in range(N_CORES):
            if tok[c] >= T_PER_CORE:
                continue
            key = (cnt[c, a] + cnt[c, b], max(cnt[c, a], cnt[c, b]), tok[c])
            if bkey is None or key < bkey:
                bkey, best = key, c
        assign[t] = best
        cnt[best, a] += 1
        cnt[best, b] += 1
        tok[best] += 1
    # per-expert segment width: max over cores, padded to 4 (uniform across
    # cores, so the SPMD program can use a different width per expert)
    caps = tuple(max(4, (int(cnt[:, e].max()) + 3) // 4 * 4) for e in range(E))
    return assign, caps


def _seg_layout(caps):
    """Segment bases, total slots, per-expert sub-tile counts, cv col bases."""
    bases = [0]
    for e in range(E):
        bases.append(bases[-1] + caps[e])
    S = bases[-1]
    nsubs = [math.ceil(caps[e] / P) for e in range(E)]
    cvb = [0]
    for e in range(E):
        cvb.append(cvb[-1] + nsubs[e])
    return bases, S, nsubs, cvb


def _build_core_inputs(x_flat_bf, e1, e2, c1, c2, assign, caps, core):
    """Slot layout + device input arrays for one core."""
    bases, S, nsubs, cvb = _seg_layout(caps)
    tokens = np.nonzero(assign == core)[0]
    assert len(tokens) == T_PER_CORE

    slot_tok = np.full(S, -1, np.int64)
    cvals = np.zeros(S, np.float32)
    slotA = np.full(T_PER_CORE, 0, np.int64)
    slotB = np.full(T_PER_CORE, 0, np.int64)
    fill = np.zeros(E, np.int64)
    for i in range(T_PER_CORE):
        g = tokens[i]
        for which, (e, c) in enumerate(((e1[g], c1[g]), (e2[g], c2[g]))):
            s = bases[int(e)] + fill[e]
            fill[e] += 1
            slot_tok[s] = i
            cvals[s] = c
            if which == 0:
                slotA[i] = s
            else:
                slotB[i] = s
    assert all(fill[e] <= caps[e] for e in range(E))

    # xg pre-tiled to the SBUF layout [p, ko, slot] so the load is contiguous
    xgT = np.zeros((H, S), NP_BF16)
    valid = slot_tok >= 0
    xgT[:, valid] = x_flat_bf[tokens[slot_tok[valid]]].T
    xgp = np.ascontiguousarray(
        xgT.reshape(H // P, P, S).transpose(1, 0, 2))

    # cv: [P, sum(nsubs)]; column cvb[e]+j holds cvals[bases[e] + j*128 : +128]
    cv = np.zeros((P, cvb[-1]), np.float32)
    for e in range(E):
        for j in range(nsubs[e]):
            w = min(P, caps[e] - j * P)
            cv[:w, cvb[e] + j] = cvals[bases[e] + j * P : bases[e] + j * P + w]

    idxA = slotA.reshape(T_PER_CORE // P, P).T.astype(np.int32).copy()
    idxB = slotB.reshape(T_PER_CORE // P, P).T.astype(np.int32).copy()
    return dict(tokens=tokens, xgT=xgp, cv=cv, idxA=idxA, idxB=idxB)


# ----------------------------------------------------------------------------
# Device program
# ----------------------------------------------------------------------------

def build_program(caps, act_fn=None):
    """One SPMD program shared by all 8 cores; `caps[e]` is expert e's padded
    segment width (uniform across cores, runtime-derived compile-time const)."""
    if act_fn is None:
        act_fn = mybir.ActivationFunctionType.Gelu
    assert max(caps) <= 512, f"routing too imbalanced: {caps=}"
    bases, S, nsubs, cvb = _seg_layout(caps)
    W1_CHUNK = 512          # w1 columns per DMA chunk (4 m-tiles)
    W2_CHUNK = 256          # w2 columns per DMA chunk (2 h-tiles)
    NCH1 = FF // W1_CHUNK
    NCH2 = H // W2_CHUNK

    nc = bacc.Bacc("TRN2", target_bir_lowering=False, debug=False,
                   num_devices=N_CORES)

    # Weights/activations arrive pre-tiled to SBUF layout (host formats them)
    # so every DMA is a fully contiguous per-partition read.
    xgT_d = nc.dram_tensor("xgT", [P, H // P, S], BF16, kind="ExternalInput")
    w1_d = nc.dram_tensor("w1b", [E, NCH1, P, (H // P) * W1_CHUNK], BF16,
                          kind="ExternalInput")
    w2_d = nc.dram_tensor("w2b", [E, NCH2, P, (FF // P) * W2_CHUNK], BF16,
                          kind="ExternalInput")
    b1_d = nc.dram_tensor("b1f", [P, E, FF // P], F32, kind="ExternalInput")
    b2_d = nc.dram_tensor("b2f", [P, E, H // P], F32, kind="ExternalInput")
    cv_d = nc.dram_tensor("cv", [P, cvb[-1]], F32, kind="ExternalInput")
    ia_d = nc.dram_tensor("idxA", [P, T_PER_CORE // P], I32, kind="ExternalInput")
    ib_d = nc.dram_tensor("idxB", [P, T_PER_CORE // P], I32, kind="ExternalInput")
    out_d = nc.dram_tensor("out", [T_PER_CORE, H], F32, kind="ExternalOutput")
    ybuf = nc.dram_tensor("ybuf", [S, H], BF16)

    with tile.TileContext(nc) as tc:
        with (
            tc.tile_pool(name="const", bufs=1) as const_pool,
            tc.tile_pool(name="xg", bufs=1) as xg_pool,
            tc.tile_pool(name="w1", bufs=2) as w1_pool,
            tc.tile_pool(name="w2", bufs=2) as w2_pool,
            tc.tile_pool(name="hid", bufs=2) as hid_pool,
            tc.tile_pool(name="yt", bufs=2) as y_pool,
            tc.tile_pool(name="yrow", bufs=3) as yrow_pool,
            tc.tile_pool(name="gath", bufs=2) as g_pool,
            tc.tile_pool(name="ps1", bufs=3, space="PSUM") as ps1_pool,
            tc.tile_pool(name="ps2", bufs=3, space="PSUM") as ps2_pool,
            tc.tile_pool(name="pst", bufs=2, space="PSUM") as pst_pool,
        ):
            identity = const_pool.tile([P, P], BF16)
            make_identity(nc, identity[:])
            cv_sb = const_pool.tile([P, cvb[-1]], F32)
            nc.sync.dma_start(out=cv_sb[:], in_=cv_d[:])
            ia_sb = const_pool.tile([P, T_PER_CORE // P], I32)
            nc.sync.dma_start(out=ia_sb[:], in_=ia_d[:])
            ib_sb = const_pool.tile([P, T_PER_CORE // P], I32)
            nc.sync.dma_start(out=ib_sb[:], in_=ib_d[:])
            b1_sb = const_pool.tile([P, E, FF // P], F32)
            nc.sync.dma_start(out=b1_sb[:], in_=b1_d[:])
            b2_sb = const_pool.tile([P, E, H // P], F32)
            nc.sync.dma_start(out=b2_sb[:], in_=b2_d[:])

            xg_sb = xg_pool.tile([P, H // P, S], BF16)
            nc.sync.dma_start(out=xg_sb[:], in_=xgT_d[:])

            for e in range(E):
                cap = caps[e]
                seg = slice(bases[e], bases[e] + cap)
                # ---- mm1: hidden^T = gelu(w1[e].T @ xgT_seg + b1) ----
                hid = hid_pool.tile([P, FF // P, cap], BF16, tag="hid")
                for mc in range(NCH1):
                    w1t = w1_pool.tile([P, H // P, W1_CHUNK], BF16)
                    nc.sync.dma_start(
                        out=w1t[:],
                        in_=w1_d[e, mc].rearrange(
                            "p (ko m) -> p ko m", ko=H // P))
                    for mi in range(W1_CHUNK // P):
                        m = mc * (W1_CHUNK // P) + mi
                        ps = ps1_pool.tile([P, cap], F32)
                        for k in range(H // P):
                            nc.tensor.matmul(
                                ps[:],
                                lhsT=w1t[:, k, mi * P:(mi + 1) * P],
                                rhs=xg_sb[:, k, seg],
                                start=(k == 0),
                                stop=(k == H // P - 1),
                            )
                        nc.scalar.activation(
                            hid[:, m, :], ps[:], act_fn,
                            bias=b1_sb[:, e, m:m + 1])

                # ---- mm2: y^T = w2[e].T @ hidden^T + b2 ----
                y_sb = y_pool.tile([P, H // P, cap], BF16, tag="y")
                for hc in range(NCH2):
                    w2t = w2_pool.tile([P, FF // P, W2_CHUNK], BF16)
                    nc.sync.dma_start(
                        out=w2t[:],
                        in_=w2_d[e, hc].rearrange(
                            "p (ko n) -> p ko n", ko=FF // P))
                    for hi in range(W2_CHUNK // P):
                        h = hc * (W2_CHUNK // P) + hi
                        ps = ps2_pool.tile([P, cap], F32)
                        for k in range(FF // P):
                            nc.tensor.matmul(
                                ps[:],
                                lhsT=w2t[:, k, hi * P:(hi + 1) * P],
                                rhs=hid[:, k, :],
                                start=(k == 0),
                                stop=(k == FF // P - 1),
                            )
                        nc.vector.tensor_scalar_add(
                            y_sb[:, h, :], ps[:], b2_sb[:, e, h:h + 1])

                # ---- transpose to token-major, scale by combine, store ----
                for j in range(nsubs[e]):
                    w = min(P, cap - j * P)
                    yrow = yrow_pool.tile([P, H], BF16)
                    for h in range(H // P):
                        pt = pst_pool.tile([P, P], BF16)
                        nc.tensor.transpose(
                            pt[:w, :], y_sb[:, h, j * P:j * P + w], identity[:])
                        nc.vector.tensor_tensor(
                            out=yrow[:w, h * P:(h + 1) * P],
                            in0=pt[:w, :],
                            in1=cv_sb[:w, cvb[e] + j:cvb[e] + j + 1]
                                .to_broadcast([w, P]),
                            op=mybir.AluOpType.mult)
                    base = bases[e] + j * P
                    nc.sync.dma_start(out=ybuf[base:base + w, :], in_=yrow[:w, :])

            # ---- combine: out[t] = ybuf[slotA[t]] + ybuf[slotB[t]] ----
            for jt in range(T_PER_CORE // P):
                gA = g_pool.tile([P, H], BF16, tag="gA")
                gB = g_pool.tile([P, H], BF16, tag="gB")
                ot = g_pool.tile([P, H], F32, tag="ot")
                nc.gpsimd.indirect_dma_start(
                    out=gA[:], out_offset=None, in_=ybuf[:],
                    in_offset=IndirectOffsetOnAxis(ap=ia_sb[:, jt:jt + 1], axis=0))
                nc.gpsimd.indirect_dma_start(
                    out=gB[:], out_offset=None, in_=ybuf[:],
                    in_offset=IndirectOffsetOnAxis(ap=ib_sb[:, jt:jt + 1], axis=0))
                nc.vector.tensor_tensor(out=ot[:], in0=gA[:], in1=gB[:],
                                        op=mybir.AluOpType.add)
                nc.sync.dma_start(out=out_d[jt * P:(jt + 1) * P, :], in_=ot[:])

    nc.compile()
    return nc


# ----------------------------------------------------------------------------
# Entry point
# ----------------------------------------------------------------------------

def prepare(x, router_w, router_b, w1, b1, w2, b2):
    """Host-side sharding: returns (nc, in_maps, per-core token lists)."""
    x_flat = np.ascontiguousarray(np.asarray(x, np.float32).reshape(T, H))
    e1, e2, c1, c2 = _route(x_flat, np.asarray(router_w), np.asarray(router_b))
    assign, caps = _assign_tokens(e1, e2)

    x_flat_bf = x_flat.astype(NP_BF16)
    W1_CHUNK, W2_CHUNK = 512, 256
    # pre-tile weights to the SBUF slab layout: [e, chunk, p, ko*chunk_cols]
    w1b = np.ascontiguousarray(
        np.asarray(w1, np.float32).astype(NP_BF16)
        .reshape(E, H // P, P, FF // W1_CHUNK, W1_CHUNK)
        .transpose(0, 3, 2, 1, 4)
        .reshape(E, FF // W1_CHUNK, P, (H // P) * W1_CHUNK))
    w2b = np.ascontiguousarray(
        np.asarray(w2, np.float32).astype(NP_BF16)
        .reshape(E, FF // P, P, H // W2_CHUNK, W2_CHUNK)
        .transpose(0, 3, 2, 1, 4)
        .reshape(E, H // W2_CHUNK, P, (FF // P) * W2_CHUNK))
    b1f = np.ascontiguousarray(
        np.asarray(b1, np.float32).reshape(E, FF // P, P).transpose(2, 0, 1))
    b2f = np.ascontiguousarray(
        np.asarray(b2, np.float32).reshape(E, H // P, P).transpose(2, 0, 1))

    in_maps = []
    token_lists = []
    for c in range(N_CORES):
        cd = _build_core_inputs(x_flat_bf, e1, e2, c1, c2, assign, caps, c)
        in_maps.append(dict(xgT=cd["xgT"], w1b=w1b, w2b=w2b, b1f=b1f, b2f=b2f,
                            cv=cd["cv"], idxA=cd["idxA"], idxB=cd["idxB"]))
        token_lists.append(cd["tokens"])

    if caps not in _PROGRAM_CACHE:
        _PROGRAM_CACHE[caps] = build_program(caps)
    return _PROGRAM_CACHE[caps], in_maps, token_lists


def kernel(x, router_w, router_b, w1, b1, w2, b2):
    nc, in_maps, token_lists = prepare(x, router_w, router_b, w1, b1, w2, b2)
    res = run_bass_kernel_spmd(nc, in_maps, core_ids=list(range(N_CORES)))
    out_full = np.empty((T, H), np.float32)
    for c in range(N_CORES):
        out_full[token_lists[c]] = res.results[c]["out"]
    return out_full.reshape(B, SEQ, H)



# revision 18
# speedup vs baseline: 1.1359x; 1.1359x over previous
"""MoE layer (nn_MoELayer_81630148428171) as a Trainium2 Bass kernel on 8 NeuronCores.

Strategy (data-parallel tokens + streamed expert weights, sparse top-2 compute):
  - Router runs on host (jax-cpu, bitwise-identical ops to the reference) and
    determines the *sharding*: tokens are assigned to the 8 cores with a greedy
    balancer so that every (core, expert) token count is ~equal; each core gets
    exactly 1024 tokens.
  - Each core receives its tokens pre-gathered into per-expert column segments
    (feature-major, bf16) and runs all 8 experts' FFN on just its routed tokens
    (top-2 sparse => ~2048 token-expert pairs per core):
        hidden^T = gelu(w1[e].T @ xgT_seg + b1)   (PE matmuls, bf16 in, f32 acc)
        y^T      = w2[e].T @ hidden^T + b2
    y^T tiles are PE-transposed to token-major and scaled by the combine
    weight; each token's two expert contributions are then merged by indirect
    scatter DMA straight into the output rows: the token's first (lower-index)
    expert writes, the second accumulates (SWDGE fp32 add) — so the combine
    overlaps the expert loop and there is no serial gather tail.  Each y tile
    is scattered twice with masked offset tables (A = overwrite pass, B =
    accumulate pass; out-of-range indices drop the other class's rows),
    because the scatter source must start at SBUF partition 0.
  - No collectives: the host concatenates the 8 disjoint per-core token slices.
"""

import math
import numpy as np
import ml_dtypes

import concourse.bacc as bacc
import concourse.mybir as mybir
import concourse.tile as tile
from concourse.bass import IndirectOffsetOnAxis
from concourse.bass_utils import run_bass_kernel_spmd
from concourse.masks import make_identity

# Problem shapes (hardcoded per contract).
B, SEQ, H = 4, 2048, 1024
T = B * SEQ
FF = 4 * H
E = 8
TOP_K = 2
N_CORES = 8
T_PER_CORE = T // N_CORES
P = 128

BF16 = mybir.dt.bfloat16
F32 = mybir.dt.float32
I32 = mybir.dt.int32
NP_BF16 = ml_dtypes.bfloat16

PAD_IDX = 1 << 20  # scatter offsets >= bounds_check are silently dropped

_PROGRAM_CACHE: dict[tuple, object] = {}


# ----------------------------------------------------------------------------
# Host-side routing + sharding
# ----------------------------------------------------------------------------

def _route(x_flat, router_w, router_b):
    """Top-2 routing with bitwise-identical math to the jax reference."""
    try:
        import jax
        import jax.numpy as jnp

        cpu = jax.devices("cpu")[0]

        def f(xf, w, b):
            logits = xf @ w + b
            probs = jax.nn.softmax(logits, axis=-1)
            top_values, top_indices = jax.lax.top_k(probs, TOP_K)
            top_values = top_values / jnp.sum(top_values, axis=-1,
                                              keepdims=True)
            return top_values, top_indices

        with jax.default_device(cpu):
            tv, ti = jax.jit(f)(
                jnp.asarray(x_flat), jnp.asarray(router_w),
                jnp.asarray(router_b))
        tv = np.asarray(tv)
        ti = np.asarray(ti)
    except Exception:
        # numpy fallback (f32, same tie-breaking as lax.top_k for distinct
        # values — differences only possible for exact float ties)
        logits = x_flat @ router_w + router_b
        p = np.exp(logits - logits.max(-1, keepdims=True))
        p /= p.sum(-1, keepdims=True)
        ti = np.argsort(-p, axis=-1, kind="stable")[:, :TOP_K]
        tv = np.take_along_axis(p, ti, axis=-1)
        tv = tv / tv.sum(-1, keepdims=True)
    return (
        ti[:, 0].astype(np.int64),
        ti[:, 1].astype(np.int64),
        tv[:, 0].astype(np.float32),
        tv[:, 1].astype(np.float32),
    )


def _assign_tokens(e1, e2):
    """Greedy balanced assignment of tokens to cores.

    Keeps per-(core, expert) slot counts nearly equal while giving every core
    exactly T_PER_CORE tokens.
    """
    cnt = np.zeros((N_CORES, E), np.int64)
    tok = np.zeros(N_CORES, np.int64)
    assign = np.empty(T, np.int64)
    for t in range(T):
        a, b = e1[t], e2[t]
        best = -1
        bkey = None
        for c in range(N_CORES):
            if tok[c] >= T_PER_CORE:
                continue
            key = (cnt[c, a] + cnt[c, b], max(cnt[c, a], cnt[c, b]), tok[c])
            if bkey is None or key < bkey:
                bkey, best = key, c
        assign[t] = best
        cnt[best, a] += 1
        cnt[best, b] += 1
        tok[best] += 1
    # per-expert segment width: max over cores, padded to 4 (uniform across
    # cores, so the SPMD program can use a different width per expert)
    caps = tuple(max(4, (int(cnt[:, e].max()) + 3) // 4 * 4) for e in range(E))
    return assign, caps


def _seg_layout(caps):
    """Segment bases, total slots, per-expert sub-tile counts, cv col bases."""
    bases = [0]
    for e in range(E):
        bases.append(bases[-1] + caps[e])
    S = bases[-1]
    nsubs = [math.ceil(caps[e] / P) for e in range(E)]
    cvb = [0]
    for e in range(E):
        cvb.append(cvb[-1] + nsubs[e])
    return bases, S, nsubs, cvb


def _build_core_inputs(x_flat_bf, e1, e2, c1, c2, assign, caps, core):
    """Slot layout + device input arrays for one core.

    A token's A slot lives in its lower-index expert's segment, the B slot in
    the higher-index one.  The device scatters every y tile twice straight
    into the output rows — once with the A offset table (overwrite), once
    with the B table (fp32 accumulate); each table masks the other class's
    slots (and padding) with out-of-range indices that the scatter bounds
    check drops.  Expert order then guarantees A lands before B per token.
    """
    bases, S, nsubs, cvb = _seg_layout(caps)
    tokens = np.nonzero(assign == core)[0]
    assert len(tokens) == T_PER_CORE

    slot_tok = np.full(S, -1, np.int64)
    cvals = np.zeros(S, np.float32)
    oa = np.full(S, PAD_IDX, np.int64)
    ob = np.full(S, PAD_IDX, np.int64)
    fill = np.zeros(E, np.int64)
    for i in range(T_PER_CORE):
        g = tokens[i]
        pairs = ((int(e1[g]), c1[g]), (int(e2[g]), c2[g]))
        (ea, ca), (eb, cb) = sorted(pairs, key=lambda p: p[0])
        for cls, (e, c) in (("a", (ea, ca)), ("b", (eb, cb))):
            s = bases[e] + fill[e]
            fill[e] += 1
            slot_tok[s] = i
            cvals[s] = c
            (oa if cls == "a" else ob)[s] = i
    assert all(fill[e] <= caps[e] for e in range(E))

    # xg pre-tiled to the SBUF layout [p, ko, slot] so the load is contiguous
    xgT = np.zeros((H, S), NP_BF16)
    valid = slot_tok >= 0
    xgT[:, valid] = x_flat_bf[tokens[slot_tok[valid]]].T
    xgp = np.ascontiguousarray(
        xgT.reshape(H // P, P, S).transpose(1, 0, 2))

    # cv/otA/otB: [P, sum(nsubs)]; column cvb[e]+j holds slot-metadata for
    # slots bases[e]+j*128 .. +128 (combine weight / output-row indices)
    cv = np.zeros((P, cvb[-1]), np.float32)
    ota = np.full((P, cvb[-1]), PAD_IDX, np.int32)
    otb = np.full((P, cvb[-1]), PAD_IDX, np.int32)
    for e in range(E):
        for j in range(nsubs[e]):
            w = min(P, caps[e] - j * P)
            sl = slice(bases[e] + j * P, bases[e] + j * P + w)
            cv[:w, cvb[e] + j] = cvals[sl]
            ota[:w, cvb[e] + j] = oa[sl]
            otb[:w, cvb[e] + j] = ob[sl]
    return dict(tokens=tokens, xgT=xgp, cv=cv, ota=ota, otb=otb)


# ----------------------------------------------------------------------------
# Device program
# ----------------------------------------------------------------------------

def build_program(caps, act_fn=None):
    """One SPMD program shared by all 8 cores; `caps[e]` is expert e's padded
    segment width (uniform across cores, runtime-derived compile-time
    const)."""
    if act_fn is None:
        act_fn = mybir.ActivationFunctionType.Gelu
    assert max(caps) <= 512, f"routing too imbalanced: {caps=}"
    bases, S, nsubs, cvb = _seg_layout(caps)
    W1_CHUNK = 512          # w1 columns per DMA chunk (4 m-tiles)
    W2_CHUNK = 256          # w2 columns per DMA chunk (2 h-tiles)
    NCH1 = FF // W1_CHUNK
    NCH2 = H // W2_CHUNK

    nc = bacc.Bacc("TRN2", target_bir_lowering=False, debug=False,
                   num_devices=N_CORES)

    # Weights/activations arrive pre-tiled to SBUF layout (host formats them)
    # so every DMA is a fully contiguous per-partition read.
    xgT_d = nc.dram_tensor("xgT", [P, H // P, S], BF16, kind="ExternalInput")
    w1_d = nc.dram_tensor("w1b", [E, NCH1, P, (H // P) * W1_CHUNK], BF16,
                          kind="ExternalInput")
    w2_d = nc.dram_tensor("w2b", [E, NCH2, P, (FF // P) * W2_CHUNK], BF16,
                          kind="ExternalInput")
    b1_d = nc.dram_tensor("b1f", [P, E, FF // P], F32, kind="ExternalInput")
    b2_d = nc.dram_tensor("b2f", [P, E, H // P], F32, kind="ExternalInput")
    cv_d = nc.dram_tensor("cv", [P, cvb[-1]], F32, kind="ExternalInput")
    ota_d = nc.dram_tensor("ota", [P, cvb[-1]], I32, kind="ExternalInput")
    otb_d = nc.dram_tensor("otb", [P, cvb[-1]], I32, kind="ExternalInput")
    out_d = nc.dram_tensor("out", [T_PER_CORE, H], F32, kind="ExternalOutput")

    with tile.TileContext(nc) as tc:
        with (
            tc.tile_pool(name="const", bufs=1) as const_pool,
            tc.tile_pool(name="xg", bufs=1) as xg_pool,
            tc.tile_pool(name="w1", bufs=3) as w1_pool,
            tc.tile_pool(name="w2", bufs=2) as w2_pool,
            tc.tile_pool(name="hid", bufs=2) as hid_pool,
            tc.tile_pool(name="yt", bufs=2) as y_pool,
            tc.tile_pool(name="yrow", bufs=3) as yrow_pool,
            tc.tile_pool(name="ps1", bufs=3, space="PSUM") as ps1_pool,
            tc.tile_pool(name="ps2", bufs=3, space="PSUM") as ps2_pool,
            tc.tile_pool(name="pst", bufs=2, space="PSUM") as pst_pool,
        ):
            identity = const_pool.tile([P, P], BF16)
            make_identity(nc, identity[:])
            cv_sb = const_pool.tile([P, cvb[-1]], F32)
            nc.sync.dma_start(out=cv_sb[:], in_=cv_d[:])
            ota_sb = const_pool.tile([P, cvb[-1]], I32)
            nc.sync.dma_start(out=ota_sb[:], in_=ota_d[:])
            otb_sb = const_pool.tile([P, cvb[-1]], I32)
            nc.sync.dma_start(out=otb_sb[:], in_=otb_d[:])
            b1_sb = const_pool.tile([P, E, FF // P], F32)
            nc.sync.dma_start(out=b1_sb[:], in_=b1_d[:])
            b2_sb = const_pool.tile([P, E, H // P], F32)
            nc.sync.dma_start(out=b2_sb[:], in_=b2_d[:])

            # per-expert xg segment tiles: expert e's matmuls only wait for
            # their own slice of the token activations (fast start).
            xg_sb = []
            for e in range(E):
                t_ = xg_pool.tile([P, H // P, caps[e]], BF16, tag=f"xg{e}")
                nc.sync.dma_start(
                    out=t_[:], in_=xgT_d[:, :, bases[e]:bases[e] + caps[e]])
                xg_sb.append(t_)

            for e in range(E):
                cap = caps[e]
                # ---- mm1: hidden^T = gelu(w1[e].T @ xgT_seg + b1) ----
                hid = hid_pool.tile([P, FF // P, cap], BF16, tag="hid")
                for mc in range(NCH1):
                    w1t = w1_pool.tile([P, H // P, W1_CHUNK], BF16)
                    nc.sync.dma_start(
                        out=w1t[:],
                        in_=w1_d[e, mc].rearrange(
                            "p (ko m) -> p ko m", ko=H // P))
                    for mi in range(W1_CHUNK // P):
                        m = mc * (W1_CHUNK // P) + mi
                        ps = ps1_pool.tile([P, cap], F32)
                        for k in range(H // P):
                            nc.tensor.matmul(
                                ps[:],
                                lhsT=w1t[:, k, mi * P:(mi + 1) * P],
                                rhs=xg_sb[e][:, k, :],
                                start=(k == 0),
                                stop=(k == H // P - 1),
                            )
                        nc.scalar.activation(
                            hid[:, m, :], ps[:], act_fn,
                            bias=b1_sb[:, e, m:m + 1])

                # ---- mm2: y^T = w2[e].T @ hidden^T + b2 ----
                y_sb = y_pool.tile([P, H // P, cap], BF16, tag="y")
                for hc in range(NCH2):
                    w2t = w2_pool.tile([P, FF // P, W2_CHUNK], BF16)
                    nc.sync.dma_start(
                        out=w2t[:],
                        in_=w2_d[e, hc].rearrange(
                            "p (ko n) -> p ko n", ko=FF // P))
                    for hi in range(W2_CHUNK // P):
                        h = hc * (W2_CHUNK // P) + hi
                        ps = ps2_pool.tile([P, cap], F32)
                        for k in range(FF // P):
                            nc.tensor.matmul(
                                ps[:],
                                lhsT=w2t[:, k, hi * P:(hi + 1) * P],
                                rhs=hid[:, k, :],
                                start=(k == 0),
                                stop=(k == FF // P - 1),
                            )
                        nc.vector.tensor_scalar_add(
                            y_sb[:, h, :], ps[:], b2_sb[:, e, h:h + 1])

                # ---- transpose to token-major, scale by combine weight,
                #      scatter into out rows (A: overwrite, B: fp32 acc) ----
                for j in range(nsubs[e]):
                    w = min(P, cap - j * P)
                    col = cvb[e] + j
                    yrow = yrow_pool.tile([P, H], F32)
                    for h in range(H // P):
                        pt = pst_pool.tile([P, P], BF16)
                        nc.tensor.transpose(
                            pt[:w, :], y_sb[:, h, j * P:j * P + w], identity[:])
                        nc.vector.tensor_tensor(
                            out=yrow[:w, h * P:(h + 1) * P],
                            in0=pt[:w, :],
                            in1=cv_sb[:w, col:col + 1].to_broadcast([w, P]),
                            op=mybir.AluOpType.mult)
                    nc.gpsimd.indirect_dma_start(
                        out=out_d[:], in_=yrow[0:w, :],
                        out_offset=IndirectOffsetOnAxis(
                            ap=ota_sb[0:w, col:col + 1], axis=0),
                        in_offset=None,
                        bounds_check=T_PER_CORE - 1, oob_is_err=False)
                    nc.gpsimd.indirect_dma_start(
                        out=out_d[:], in_=yrow[0:w, :],
                        out_offset=IndirectOffsetOnAxis(
                            ap=otb_sb[0:w, col:col + 1], axis=0),
                        in_offset=None,
                        bounds_check=T_PER_CORE - 1, oob_is_err=False,
                        compute_op=mybir.AluOpType.add)

    nc.compile()
    return nc


# ----------------------------------------------------------------------------
# Entry point
# ----------------------------------------------------------------------------

def prepare(x, router_w, router_b, w1, b1, w2, b2):
    """Host-side sharding: returns (nc, in_maps, per-core token lists)."""
    x_flat = np.ascontiguousarray(np.asarray(x, np.float32).reshape(T, H))
    e1, e2, c1, c2 = _route(x_flat, np.asarray(router_w), np.asarray(router_b))
    assign, caps = _assign_tokens(e1, e2)

    x_flat_bf = x_flat.astype(NP_BF16)
    W1_CHUNK, W2_CHUNK = 512, 256
    # pre-tile weights to the SBUF slab layout: [e, chunk, p, ko*chunk_cols]
    w1b = np.ascontiguousarray(
        np.asarray(w1, np.float32).astype(NP_BF16)
        .reshape(E, H // P, P, FF // W1_CHUNK, W1_CHUNK)
        .transpose(0, 3, 2, 1, 4)
        .reshape(E, FF // W1_CHUNK, P, (H // P) * W1_CHUNK))
    w2b = np.ascontiguousarray(
        np.asarray(w2, np.float32).astype(NP_BF16)
        .reshape(E, FF // P, P, H // W2_CHUNK, W2_CHUNK)
        .transpose(0, 3, 2, 1, 4)
        .reshape(E, H // W2_CHUNK, P, (FF // P) * W2_CHUNK))
    b1f = np.ascontiguousarray(
        np.asarray(b1, np.float32).reshape(E, FF // P, P).transpose(2, 0, 1))
    b2f = np.ascontiguousarray(
        np.asarray(b2, np.float32).reshape(E, H // P, P).transpose(2, 0, 1))

    in_maps = []
    token_lists = []
    for c in range(N_CORES):
        cd = _build_core_inputs(x_flat_bf, e1, e2, c1, c2, assign, caps, c)
        in_maps.append(dict(xgT=cd["xgT"], w1b=w1b, w2b=w2b, b1f=b1f, b2f=b2f,
                            cv=cd["cv"], ota=cd["ota"], otb=cd["otb"]))
        token_lists.append(cd["tokens"])

    if caps not in _PROGRAM_CACHE:
        _PROGRAM_CACHE[caps] = build_program(caps)
    return _PROGRAM_CACHE[caps], in_maps, token_lists


def kernel(x, router_w, router_b, w1, b1, w2, b2):
    nc, in_maps, token_lists = prepare(x, router_w, router_b, w1, b1, w2, b2)
    res = run_bass_kernel_spmd(nc, in_maps, core_ids=list(range(N_CORES)))
    out_full = np.empty((T, H), np.float32)
    for c in range(N_CORES):
        out_full[token_lists[c]] = res.results[c]["out"]
    return out_full.reshape(B, SEQ, H)
